# revision 43
# baseline (speedup 1.0000x reference)
"""Trainium2 Bass kernel for GQA attention block (B=2, S=2048, H=2048,
16 q-heads / 4 kv-heads, head_dim=128, RoPE, causal) on 8 NeuronCores.

Sharding: core c -> batch b = c // 4, kv-group g = c % 4
  (q heads 4g..4g+3, kv head g).  Each core computes its batch's
  attention for its 4 query heads plus the partial output projection
  over its 512 hidden columns of w_o; host sums the 4 partials per batch.

Design (all matmuls bf16, PSUM accumulate f32):
  Phase 1  QKV projection; half 0: chunk c2=0 runs as a 6-chain wavefront
           matched to the DMA arrival order (first w tile split so the
           first matmul starts after ~100KB of DMA); chunk c2=1 runs
           chain-outer with the k/v chains FIRST so their consumes (kT
           RoPE add, v transposes into vaug) are complete before the PE
           drains the remaining q chains -- attention then starts with no
           stall.  Half 1 is emitted lazily as PE *filler* inside the
           qb0/qb1 attention loops (i-outer, 2 psum bufs, 4-matmul
           segments).  RoPE rotate-half via a PE permutation matmul, v
           transposed on PE into vaug blocks (136-col stride; col 128 =
           ones).
  Phase 2  per query block (512) x head: scores^T [sk part, sq free],
           exp on scalar engine, causal diagonal as a post-exp 0/1
           triangle multiply on gpsimd, then PV with e as stationary
           operand: out[sq, 129] = e^T @ [v | 1] -- the 129th column is
           the softmax denominator (free).  PV psum padded to a full
           2KB bank (sub-bank psum slots share a bank and matmul
           start=True clears the whole bank's has_written bits).
           Per-partition reciprocal + tensor_scalar normalize, DMA-xbar
           transpose back to attnT [d, s].
  Phase 3  o-proj chunks drained by a deficit pacer (emitted-PE-ns vs
           emitted-scalar-ns clocks) into later query blocks' attention
           so the PE never starves while the scalar engine churns exps.
           The four 512-col chunks of each output row-block accumulate
           in one SBUF row tile and leave in a single [128, 2048] DMA
           (4x fewer out-DMAs -- each DMA instruction costs ~600ns of
           serialized HWDGE plus ~900ns of semaphore propagation).
"""

import contextlib
import math
import numpy as np
import ml_dtypes

import concourse.bacc as bacc
import concourse.mybir as mybir
import concourse.tile as tile
from concourse.bass_utils import run_bass_kernel_spmd

F32 = mybir.dt.float32
BF16 = mybir.dt.bfloat16
AF = mybir.ActivationFunctionType

S = 2048
H = 2048
D = 128            # head dim
KT = 16            # contraction tiles over hidden (2048/128)
NQ = 512           # query block width in attention
SCALE = 1.0 / math.sqrt(D)

_CACHED = {}
MARKS = []  # (label, instruction-counter) build-time phase markers


def _mark(nc, label):
    MARKS.append((label, len(nc._state.inst_map)))


def build_nc(loop_n=None):
    MARKS.clear()
    nc = bacc.Bacc(None, target_bir_lowering=False)
    hT = nc.dram_tensor("hT", [H, S], BF16, kind="ExternalInput")
    wqk = nc.dram_tensor("wqk", [6, 128, KT * 128], BF16, kind="ExternalInput")
    cosT = nc.dram_tensor("cosT", [D, S], BF16, kind="ExternalInput")
    sinT = nc.dram_tensor("sinT", [D, S], F32, kind="ExternalInput")
    rotp = nc.dram_tensor("rotp", [128, 128], BF16, kind="ExternalInput")
    identb = nc.dram_tensor("identb", [128, 128], BF16, kind="ExternalInput")
    trimask = nc.dram_tensor("trimask", [128, 128], BF16, kind="ExternalInput")
    wo = nc.dram_tensor("wo", [4, 128, H], BF16, kind="ExternalInput")
    out = nc.dram_tensor("out", [S, H], BF16, kind="ExternalOutput")

    with tile.TileContext(nc) as tc:
        with tc.tile_pool(name="persist", bufs=1) as pp:
          with (tc.For_i(0, loop_n, 1) if loop_n else contextlib.nullcontext()):
            # ---- persistent tiles ----
            qk = [pp.tile([128, S], BF16, name=f"qk{i}", tag=f"qk{i}") for i in range(5)]
            # v blocks padded to 136 cols so each block start is 16B-aligned
            # (the DMA xbar transpose writes require it); col 128 = ones
            vaug = pp.tile([128, 16 * 136], BF16, tag="vaug")
            cos_sb = pp.tile([128, S], BF16, tag="cos")
            sin_sb = pp.tile([128, S], F32, tag="sin")
            rotp_sb = pp.tile([128, 128], BF16, tag="rotp")
            ident_sb = pp.tile([128, 128], BF16, tag="ident")
            tri_sb = pp.tile([128, 128], BF16, tag="tri")
            attnT = [
                pp.tile([128, S], BF16, name=f"at{h}", tag=f"at{h}") for h in range(4)
            ]
            wo_sb = [
                pp.tile([128, H], BF16, name=f"wo{kb}", tag=f"wo{kb}") for kb in range(4)
            ]

            nc.vector.memset(vaug[:, 128::136], 1.0)
            # warm the scalar engine's exp table while the PE waits on DMA
            scratch = pp.tile([1, 8], F32, tag="scratch")
            nc.vector.memset(scratch[:], 0.0)
            nc.scalar.activation(scratch[:], scratch[:], AF.Exp, scale=1.0)

            # ---- SBUF pools spanning both phases ----
            with (
                tc.tile_pool(name="ht", bufs=1) as htp,
                tc.tile_pool(name="wq", bufs=1) as wqp,
                tc.tile_pool(name="p1sb", bufs=2) as sb1,
                tc.tile_pool(name="epool", bufs=10) as ep,  # e2 bufs set per-call
                tc.tile_pool(name="small", bufs=2) as sp,
            ):
                w_sb = [
                    wqp.tile([128, KT * 128], BF16, name=f"w{i}", tag=f"w{i}")
                    for i in range(6)
                ]
                ht0 = [
                    htp.tile([128, 1024], BF16, name=f"ht{kt}", tag=f"ht{kt}")
                    for kt in range(KT)
                ]
                ht1 = [
                    htp.tile([128, 1024], BF16, name=f"htb{kt}", tag=f"htb{kt}")
                    for kt in range(KT)
                ]
                # DMA queue: half-0 feed (weights interleaved in consumption
                # order), then non-PE-blocking small tiles, then half-1 ht
                # (prefetch during half-0 compute), then wo.
                arrival = []
                # first two kt-tiles of w0 alone so the first matmul can
                # start after ~100KB of DMA instead of ~380KB
                nc.sync.dma_start(out=w_sb[0][:, : 2 * 128], in_=wqk[0][:, : 2 * 128])
                arrival.append(("w2", 0))
                nc.sync.dma_start(
                    out=w_sb[0][:, 2 * 128 : 8 * 128], in_=wqk[0][:, 2 * 128 : 8 * 128]
                )
                arrival.append(("wlo", 0))
                for kt in range(KT):
                    # first 512 columns only -- all chunk-0 needs; the rest
                    # streams during chunk-0 compute
                    nc.sync.dma_start(
                        out=ht0[kt][:, 0:512], in_=hT[kt * 128 : (kt + 1) * 128, 0:512]
                    )
                    arrival.append(("ht", kt))
                    if kt == 1:
                        nc.sync.dma_start(
                            out=w_sb[0][:, 8 * 128 :], in_=wqk[0][:, 8 * 128 :]
                        )
                        arrival.append(("whi", 0))
                    if kt % 3 == 2 and kt // 3 < 5:
                        j = kt // 3 + 1
                        nc.sync.dma_start(
                            out=w_sb[j][:, : 8 * 128], in_=wqk[j][:, : 8 * 128]
                        )
                        arrival.append(("wlo", j))
                    if kt % 3 == 0 and kt > 0 and kt // 3 < 6:
                        j = kt // 3
                        nc.sync.dma_start(
                            out=w_sb[j][:, 8 * 128 :], in_=wqk[j][:, 8 * 128 :]
                        )
                        arrival.append(("whi", j))
                for kt in range(KT):
                    nc.sync.dma_start(
                        out=ht0[kt][:, 512:1024],
                        in_=hT[kt * 128 : (kt + 1) * 128, 512:1024],
                    )
                nc.sync.dma_start(out=rotp_sb[:], in_=rotp[:])
                nc.sync.dma_start(out=ident_sb[:], in_=identb[:])
                nc.sync.dma_start(out=tri_sb[:], in_=trimask[:])
                nc.sync.dma_start(out=cos_sb[:], in_=cosT[:])
                nc.sync.dma_start(out=sin_sb[:], in_=sinT[:])
                for kt in range(KT):
                    nc.sync.dma_start(
                        out=ht1[kt][:], in_=hT[kt * 128 : (kt + 1) * 128, 1024:2048]
                    )
                for kb in range(4):
                    nc.sync.dma_start(out=wo_sb[kb][:], in_=wo[kb])

                # ---- Phase 1 half 0 ----
                # psrot spans phase 1 and the half-1 filler (manual scope:
                # closed right after the psqB block below)
                psrot_cm = tc.tile_pool(name="psrot", bufs=1, space="PSUM")
                psrot = psrot_cm.__enter__()
                psvt_cm = tc.tile_pool(name="psvt", bufs=1, space="PSUM")
                psvt = psvt_cm.__enter__()
                with tc.tile_pool(name="psqA", bufs=6, space="PSUM") as psqA:
                    for c2 in range(2):
                        _mark(nc, f'p1h0-c{c2}')
                        cs = c2 * 512
                        ps = [
                            psqA.tile([128, 512], F32, name=f"psq{i}", tag="psq")
                            for i in range(6)
                        ]

                        def mm(i, kt):
                            nc.tensor.matmul(
                                ps[i][:],
                                lhsT=w_sb[i][:, kt * 128 : (kt + 1) * 128],
                                rhs=ht0[kt][:, c2 * 512 : (c2 + 1) * 512],
                                start=(kt == 0),
                                stop=(kt == KT - 1),
                            )

                        if c2 == 0:
                            # wavefront emission matching DMA arrival order
                            # per-chain arrived-kt watermark: lo half = kt<8,
                            # hi half completes the chain
                            wlim = [0] * 6
                            akt = 0
                            emitted = [0] * 6

                            def advance():
                                for i in range(6):
                                    top = min(akt, wlim[i])
                                    for kt in range(emitted[i], top):
                                        mm(i, kt)
                                    emitted[i] = max(emitted[i], top)

                            for kind, idx in arrival:
                                if kind == "w2":
                                    wlim[idx] = 2
                                elif kind == "wlo":
                                    wlim[idx] = 8
                                elif kind == "whi":
                                    wlim[idx] = 16
                                else:
                                    akt = idx + 1
                                advance()
                            _mark(nc, f'p1h0-consume-c{c2}')
                            # k/v consumes first: attention's first scores
                            # need kT and vaug complete (tile-granular deps)
                            for i in (4, 5, 0, 1, 2, 3):
                                _consume_proj(
                                    nc, sb1, psrot, psvt, rotp_sb, ident_sb,
                                    ps[i], i, cs, qk, vaug, cos_sb, sin_sb,
                                    use_act=(i % 2 == 0),
                                )
                        else:
                            # chain-outer with k/v chains first so their
                            # consumes (kT add, vaug transposes) finish well
                            # before the PE drains the remaining q chains --
                            # the attention phase then starts with no stall
                            _mark(nc, f'p1h0-consume-c{c2}')
                            for i in (4, 5, 0, 1, 2, 3):
                                for kt in range(KT):
                                    mm(i, kt)
                                _consume_proj(
                                    nc, sb1, psrot, psvt, rotp_sb, ident_sb,
                                    ps[i], i, cs, qk, vaug, cos_sb, sin_sb,
                                    use_act=(i % 2 == 0),
                                )

                # ---- Phase 2+3 (attention; half-1 QKV and o-proj fill) ----
                if True:
                    pending = []  # o-proj chunks (sb, n) ready to emit
                    state = {"gen": None, "psop": None, "sc": None,
                             "scw": None, "pv": None}
                    clock = {"pe": 0.0, "act": 0.0}

                    def emit_oproj_chunk(final=False):
                        sb, n = pending.pop(0)
                        _mark(nc, f'oproj-{sb}-{n}')
                        pst = state["psop"].tile([128, 512], F32, tag="po")
                        for kb in range(4):
                            nc.tensor.matmul(
                                pst[:],
                                lhsT=attnT[kb][:, sb * 128 : (sb + 1) * 128],
                                rhs=wo_sb[kb][:, n * 512 : (n + 1) * 512],
                                start=(kb == 0),
                                stop=(kb == 3),
                            )
                        # accumulate the four 512-col chunks of one output
                        # row-block in SBUF, write the row with ONE DMA --
                        # chunks of an sb are popped consecutively (FIFO)
                        if n == 0:
                            state["orow"] = sp.tile(
                                [128, H], BF16, name="orow", tag="orow"
                            )
                        if final and n % 2 == 1:
                            nc.scalar.copy(
                                state["orow"][:, n * 512 : (n + 1) * 512], pst[:]
                            )
                        else:
                            nc.vector.tensor_copy(
                                state["orow"][:, n * 512 : (n + 1) * 512], pst[:]
                            )
                        if n == 3:
                            nc.sync.dma_start(
                                out=out[sb * 128 : (sb + 1) * 128, :],
                                in_=state["orow"][:],
                            )

                    def tick(pe_ns=0.0, act_ns=0.0):
                        clock["pe"] += pe_ns
                        clock["act"] += act_ns
                        while clock["act"] > clock["pe"] + 400.0:
                            if state["gen"] is not None:
                                try:
                                    clock["pe"] += next(state["gen"])
                                    continue
                                except StopIteration:
                                    state["gen"] = None
                            if pending and state["psop"] is not None:
                                emit_oproj_chunk()
                                clock["pe"] += 524.0
                                continue
                            break

                    kT = qk[4]

                    def emit_qb(qb):
                        q0 = qb * NQ
                        nj = 4 * qb + 4
                        for h in range(4):
                            _mark(nc, f'qb{qb}-h{h}')
                            qT = qk[h]
                            e_tiles = {}

                            def emit_j(j):
                                r4 = j - 4 * qb
                                off = max(0, r4) * 128
                                w = NQ - off
                                sps = state["sc"].tile([128, NQ], F32, tag="sc")
                                nc.tensor.matmul(
                                    sps[:, off:NQ],
                                    lhsT=kT[:, j * 128 : (j + 1) * 128],
                                    rhs=qT[:, q0 + off : q0 + NQ],
                                    start=True,
                                    stop=True,
                                )
                                e = ep.tile([128, NQ], BF16, tag="e")
                                nc.scalar.activation(
                                    e[:, off:NQ], sps[:, off:NQ], AF.Exp, scale=SCALE
                                )
                                if r4 >= 0:
                                    nc.gpsimd.tensor_mul(
                                        e[:, off : off + 128],
                                        e[:, off : off + 128],
                                        tri_sb[:],
                                    )
                                e_tiles[j] = e
                                tick(pe_ns=w / 4.8 + 35.0, act_ns=w * 0.8333 + 290.0)

                            def emit_j_pair(j0, j1):
                                # two full-width score tiles into one 2-bank
                                # psum (each matmul stays within one bank),
                                # ONE exp over [128, 1024] -- halves the
                                # per-instruction ACT overhead (~293ns each)
                                spw = state["scw"].tile(
                                    [128, 1024], F32, name="spw", tag="scw"
                                )
                                for k, j in enumerate((j0, j1)):
                                    nc.tensor.matmul(
                                        spw[:, k * 512 : (k + 1) * 512],
                                        lhsT=kT[:, j * 128 : (j + 1) * 128],
                                        rhs=qT[:, q0 : q0 + NQ],
                                        start=True,
                                        stop=True,
                                    )
                                e2 = ep.tile(
                                    [128, 1024], BF16, name="e2", tag="e2", bufs=7
                                )
                                nc.scalar.activation(e2[:], spw[:], AF.Exp, scale=SCALE)
                                e_tiles[j0] = e2[:, 0:512]
                                e_tiles[j1] = e2[:, 512:1024]
                                tick(
                                    pe_ns=2 * (NQ / 4.8 + 35.0),
                                    act_ns=1024 * 0.8333 + 290.0,
                                )

                            if qb >= 2 and state["scw"] is not None:
                                for a in range(0, 4 * qb, 2):
                                    emit_j_pair(a, a + 1)
                                for j in range(4 * qb, nj):
                                    emit_j(j)
                            else:
                                for j in range(nj):
                                    emit_j(j)

                            # PV: sequential per-t chains; vaug's ones column
                            # makes column 128 the softmax denominator
                            _mark(nc, f'qb{qb}-h{h}-pv')
                            rec = sp.tile([128, 4], F32, tag="rec")
                            asb = sp.tile([128, NQ], BF16, tag="asb")
                            for t in range(4):
                                # full-bank tile: psum slots pack at byte
                                # granularity, and a 516B slot would share
                                # its bank with the other buf -- start=True
                                # clears the whole bank's has_written bits,
                                # wiping the other chain's accumulation
                                pvt = state["pv"].tile([128, 512], F32, tag="pvt")
                                for j in range(4 * qb + t + 1):
                                    nc.tensor.matmul(
                                        pvt[:, 0:129],
                                        lhsT=e_tiles[j][:, t * 128 : (t + 1) * 128],
                                        rhs=vaug[:, j * 136 : j * 136 + 129],
                                        start=(j == 0),
                                        stop=(j == 4 * qb + t),
                                    )
                                tick(pe_ns=(4 * qb + t + 1) * 81.0)
                                nc.vector.reciprocal(rec[:, t : t + 1], pvt[:, 128:129])
                                nc.vector.tensor_scalar_mul(
                                    asb[:, t * 128 : (t + 1) * 128],
                                    pvt[:, 0:128],
                                    rec[:, t : t + 1],
                                )
                                # transpose on the DMA xbar instead of the PE
                                nc.sync.dma_start_transpose(
                                    attnT[h][:, q0 + t * 128 : q0 + (t + 1) * 128],
                                    asb[:, t * 128 : (t + 1) * 128],
                                )
                            e_tiles.clear()

                        for sbl in range(4):
                            for n in range(4):
                                pending.append((qb * 4 + sbl, n))

                    # qb0/qb1 share psum with half-1 QKV (i-outer, 2 bufs)
                    with (
                        tc.tile_pool(name="sps", bufs=3, space="PSUM") as spsp,
                        tc.tile_pool(name="pv", bufs=1, space="PSUM") as pvp,
                        tc.tile_pool(name="psqB", bufs=2, space="PSUM") as psqB,
                    ):
                        state["sc"] = spsp
                        state["pv"] = pvp

                        def qkv_half1():
                            for c2 in range(2):
                                cs = 1024 + c2 * 512
                                for i in range(6):
                                    _mark(nc, f'p1h1-c{c2}-i{i}')
                                    p = psqB.tile([128, 512], F32, tag="psqB")
                                    for kt0 in range(0, KT, 4):
                                        for kt in range(kt0, kt0 + 4):
                                            nc.tensor.matmul(
                                                p[:],
                                                lhsT=w_sb[i][:, kt * 128 : (kt + 1) * 128],
                                                rhs=ht1[kt][:, c2 * 512 : (c2 + 1) * 512],
                                                start=(kt == 0),
                                                stop=(kt == KT - 1),
                                            )
                                        yield 4 * 112.0
                                    _consume_proj(
                                        nc, sb1, psrot, psvt, rotp_sb, ident_sb,
                                        p, i, cs, qk, vaug, cos_sb, sin_sb,
                                        use_act=False,
                                    )
                                    yield 250.0

                        state["gen"] = qkv_half1()
                        emit_qb(0)
                        emit_qb(1)
                        g = state["gen"]
                        if g is not None:
                            for _ in g:
                                pass
                            state["gen"] = None

                psvt_cm.__exit__(None, None, None)
                psrot_cm.__exit__(None, None, None)

                # qb2/qb3: all 8 banks free -- wide 2-bank score psums for
                # the paired exps, 1 narrow for the causal-diagonal tiles
                with (
                    tc.tile_pool(name="scw", bufs=1, space="PSUM") as scwp,
                    tc.tile_pool(name="sps2", bufs=1, space="PSUM") as sps2,
                    tc.tile_pool(name="pv2", bufs=2, space="PSUM") as pvp2,
                    tc.tile_pool(name="pso", bufs=3, space="PSUM") as psop,
                ):
                    state["sc"] = sps2
                    state["scw"] = scwp
                    state["pv"] = pvp2
                    state["psop"] = psop
                    emit_qb(2)
                    emit_qb(3)
                    _mark(nc, 'final-drain')
                    while pending:
                        emit_oproj_chunk(final=True)

    nc.compile()
    return nc


def _consume_proj(nc, sb1, psrot, psvt, rotp_sb, ident_sb, p, i, cs, qk, vaug,
                  cos_sb, sin_sb, use_act=True):
    """Consume projection psum p (head-block i) for s-chunk [cs, cs+512)."""
    if i < 5:
        qraw = sb1.tile([128, 512], BF16, tag="qraw")
        # split psum-drain copies between scalar and vector engines
        if use_act:
            nc.scalar.copy(qraw[:], p[:])
        else:
            nc.vector.tensor_copy(qraw[:], p[:])
        rot = psrot.tile([128, 512], F32, tag="rot")
        nc.tensor.matmul(rot[:], lhsT=rotp_sb[:], rhs=qraw[:], start=True, stop=True)
        tmp = sb1.tile([128, 512], F32, tag="tmp")
        nc.vector.tensor_mul(tmp[:], rot[:], sin_sb[:, cs : cs + 512])
        t2 = sb1.tile([128, 512], F32, tag="t2")
        nc.gpsimd.tensor_mul(t2[:], qraw[:], cos_sb[:, cs : cs + 512])
        nc.vector.tensor_add(qk[i][:, cs : cs + 512], t2[:], tmp[:])
    else:
        vTc = sb1.tile([128, 512], BF16, tag="vTc")
        nc.vector.tensor_copy(vTc[:], p[:])
        for sbl in range(4):
            sb = cs // 128 + sbl
            pv = psvt.tile([128, 128], BF16, tag="psv")
            nc.tensor.transpose(pv[:], vTc[:, sbl * 128 : (sbl + 1) * 128], ident_sb[:])
            if use_act:
                nc.scalar.copy(vaug[:, sb * 136 : sb * 136 + 128], pv[:])
            else:
                nc.vector.tensor_copy(vaug[:, sb * 136 : sb * 136 + 128], pv[:])
    return


def _prep_inputs(hidden_states, cos, sin, w_qkv, w_o):
    """Build the 8 per-core input maps (host-side shard + transpose)."""
    bf = ml_dtypes.bfloat16
    hidden_states = np.asarray(hidden_states, dtype=np.float32)
    cos = np.asarray(cos, dtype=np.float32)
    sin = np.asarray(sin, dtype=np.float32)
    w_qkv = np.asarray(w_qkv, dtype=np.float32)
    w_o = np.asarray(w_o, dtype=np.float32)

    cosT = np.ascontiguousarray(cos.T).astype(bf)
    sinT = np.ascontiguousarray(sin.T).copy()
    sinT[0:64] *= -1.0  # rotate_half sign folded into sin

    d = np.arange(128)
    rotp = ((d[None, :] == (d[:, None] + 64) % 128)).astype(bf)
    identb = np.eye(128).astype(bf)
    sk = np.arange(128)[:, None]
    c = np.arange(128)[None, :]
    trimask = (sk <= c).astype(bf)

    hT = [np.ascontiguousarray(hidden_states[b].T).astype(bf) for b in range(2)]

    in_maps = []
    for cidx in range(8):
        b, g = divmod(cidx, 4)
        W6 = np.stack(
            [w_qkv[(4 * g + i) * 128 : (4 * g + i + 1) * 128] for i in range(4)]
            + [w_qkv[(16 + g) * 128 : (17 + g) * 128]]
            + [w_qkv[(20 + g) * 128 : (21 + g) * 128]]
        )  # [6, 128 m, 2048 h]
        wqk_pack = np.ascontiguousarray(
            W6.transpose(0, 2, 1)  # [6, h, m]
            .reshape(6, KT, 128, 128)  # [6, kt, p, m]
            .transpose(0, 2, 1, 3)  # [6, p, kt, m]
            .reshape(6, 128, KT * 128)
        ).astype(bf)
        wo_pack = np.ascontiguousarray(
            np.stack(
                [
                    w_o[:, (4 * g + kb) * 128 : (4 * g + kb + 1) * 128].T
                    for kb in range(4)
                ]
            )
        ).astype(bf)  # [4, 128 hd, 2048 o]
        in_maps.append(
            dict(
                hT=hT[b],
                wqk=wqk_pack,
                cosT=cosT,
                sinT=sinT,
                rotp=rotp,
                identb=identb,
                trimask=trimask,
                wo=wo_pack,
            )
        )
    return in_maps


def run(hidden_states, cos, sin, w_qkv, w_o, trace=False, **trace_kwargs):
    if "nc" not in _CACHED:
        _CACHED["nc"] = build_nc()
    nc = _CACHED["nc"]
    in_maps = _prep_inputs(hidden_states, cos, sin, w_qkv, w_o)
    res = run_bass_kernel_spmd(
        nc, in_maps, core_ids=list(range(8)), trace=trace, **trace_kwargs
    )
    outs = [res.results[c]["out"].astype(np.float32) for c in range(8)]
    full = np.stack(
        [
            outs[0] + outs[1] + outs[2] + outs[3],
            outs[4] + outs[5] + outs[6] + outs[7],
        ]
    ).astype(np.float32)
    return full, res


def kernel(hidden_states, cos, sin, w_qkv, w_o):
    full, _ = run(hidden_states, cos, sin, w_qkv, w_o, trace=False)
    return full



# revision 45
# speedup vs baseline: 1.0510x; 1.0510x over previous
"""Trainium2 Bass kernel for GQA attention block (B=2, S=2048, H=2048,
16 q-heads / 4 kv-heads, head_dim=128, RoPE, causal) on 8 NeuronCores.

Sharding: core c -> batch b = c // 4, kv-group g = c % 4
  (q heads 4g..4g+3, kv head g).  Each core computes its batch's
  attention for its 4 query heads plus the partial output projection
  over its 512 hidden columns of w_o; host sums the 4 partials per batch.

Design (all matmuls bf16, PSUM accumulate f32):
  Phase 1  QKV projection; half 0: chunk c2=0 runs as a 6-chain wavefront
           matched to the DMA arrival order (first w tile split so the
           first matmul starts after ~100KB of DMA); chunk c2=1 runs
           chain-outer with the k/v chains FIRST so their consumes (kT
           RoPE add, v transposes into vaug) are complete before the PE
           drains the remaining q chains -- attention then starts with no
           stall.  Half 1 is emitted lazily as PE *filler* inside the
           qb0/qb1 attention loops (i-outer, 2 psum bufs, 4-matmul
           segments).  RoPE rotate-half via a PE permutation matmul, v
           transposed on PE into vaug blocks (136-col stride; col 128 =
           ones).
  Phase 2  per query block (512) x head: scores^T [sk part, sq free],
           exp on scalar engine, causal diagonal as a post-exp 0/1
           triangle multiply on gpsimd, then PV with e as stationary
           operand: out[sq, 129] = e^T @ [v | 1] -- the 129th column is
           the softmax denominator (free).  PV psum padded to a full
           2KB bank (sub-bank psum slots share a bank and matmul
           start=True clears the whole bank's has_written bits).
           Per-partition reciprocal + tensor_scalar normalize, DMA-xbar
           transpose back to attnT [d, s].
  Phase 3  o-proj chunks drained by a deficit pacer (emitted-PE-ns vs
           emitted-scalar-ns clocks) into later query blocks' attention
           so the PE never starves while the scalar engine churns exps.
           The four 512-col chunks of each output row-block accumulate
           in one SBUF row tile and leave in a single [128, 2048] DMA
           (4x fewer out-DMAs -- each DMA instruction costs ~600ns of
           serialized HWDGE plus ~900ns of semaphore propagation).
"""

import contextlib
import math
import numpy as np
import ml_dtypes

import concourse.bacc as bacc
import concourse.mybir as mybir
import concourse.tile as tile
from concourse.bass_utils import run_bass_kernel_spmd

F32 = mybir.dt.float32
BF16 = mybir.dt.bfloat16
AF = mybir.ActivationFunctionType

S = 2048
H = 2048
D = 128            # head dim
KT = 16            # contraction tiles over hidden (2048/128)
NQ = 512           # query block width in attention
SCALE = 1.0 / math.sqrt(D)

_CACHED = {}
MARKS = []  # (label, instruction-counter) build-time phase markers


def _mark(nc, label):
    MARKS.append((label, len(nc._state.inst_map)))


def build_nc(loop_n=None):
    MARKS.clear()
    nc = bacc.Bacc(None, target_bir_lowering=False)
    hT = nc.dram_tensor("hT", [H, S], BF16, kind="ExternalInput")
    wqk = nc.dram_tensor("wqk", [6, 128, KT * 128], BF16, kind="ExternalInput")
    cosT = nc.dram_tensor("cosT", [D, S], BF16, kind="ExternalInput")
    sinT = nc.dram_tensor("sinT", [D, S], F32, kind="ExternalInput")
    rotp = nc.dram_tensor("rotp", [128, 128], BF16, kind="ExternalInput")
    identb = nc.dram_tensor("identb", [128, 128], BF16, kind="ExternalInput")
    trimask = nc.dram_tensor("trimask", [128, 128], BF16, kind="ExternalInput")
    wo = nc.dram_tensor("wo", [4, 128, H], BF16, kind="ExternalInput")
    out = nc.dram_tensor("out", [S, H], BF16, kind="ExternalOutput")

    with tile.TileContext(nc) as tc:
        with tc.tile_pool(name="persist", bufs=1) as pp:
          with (tc.For_i(0, loop_n, 1) if loop_n else contextlib.nullcontext()):
            # ---- persistent tiles ----
            qk = [pp.tile([128, S], BF16, name=f"qk{i}", tag=f"qk{i}") for i in range(5)]
            # v blocks padded to 136 cols so each block start is 16B-aligned
            # (the DMA xbar transpose writes require it); col 128 = ones
            vaug = pp.tile([128, 16 * 136], BF16, tag="vaug")
            cos_sb = pp.tile([128, S], BF16, tag="cos")
            sin_sb = pp.tile([128, S], F32, tag="sin")
            rotp_sb = pp.tile([128, 128], BF16, tag="rotp")
            ident_sb = pp.tile([128, 128], BF16, tag="ident")
            tri_sb = pp.tile([128, 128], BF16, tag="tri")
            attnT = [
                pp.tile([128, S], BF16, name=f"at{h}", tag=f"at{h}") for h in range(4)
            ]
            wo_sb = [
                pp.tile([128, H], BF16, name=f"wo{kb}", tag=f"wo{kb}") for kb in range(4)
            ]

            nc.vector.memset(vaug[:, 128::136], 1.0)
            # warm the scalar engine's exp table while the PE waits on DMA
            scratch = pp.tile([1, 8], F32, tag="scratch")
            nc.vector.memset(scratch[:], 0.0)
            nc.scalar.activation(scratch[:], scratch[:], AF.Exp, scale=1.0)

            # ---- SBUF pools spanning both phases ----
            with (
                tc.tile_pool(name="ht", bufs=1) as htp,
                tc.tile_pool(name="wq", bufs=1) as wqp,
                tc.tile_pool(name="p1sb", bufs=2) as sb1,
                tc.tile_pool(name="epool", bufs=10) as ep,  # e2 bufs set per-call
                tc.tile_pool(name="small", bufs=2) as sp,
            ):
                w_sb = [
                    wqp.tile([128, KT * 128], BF16, name=f"w{i}", tag=f"w{i}")
                    for i in range(6)
                ]
                ht0 = [
                    htp.tile([128, 1024], BF16, name=f"ht{kt}", tag=f"ht{kt}")
                    for kt in range(KT)
                ]
                ht1 = [
                    htp.tile([128, 1024], BF16, name=f"htb{kt}", tag=f"htb{kt}")
                    for kt in range(KT)
                ]
                # DMA queue: half-0 feed (weights interleaved in consumption
                # order), then non-PE-blocking small tiles, then half-1 ht
                # (prefetch during half-0 compute), then wo.
                arrival = []
                # first two kt-tiles of w0 alone so the first matmul can
                # start after ~100KB of DMA instead of ~380KB
                nc.sync.dma_start(out=w_sb[0][:, : 2 * 128], in_=wqk[0][:, : 2 * 128])
                arrival.append(("w2", 0))
                nc.sync.dma_start(
                    out=w_sb[0][:, 2 * 128 : 8 * 128], in_=wqk[0][:, 2 * 128 : 8 * 128]
                )
                arrival.append(("wlo", 0))
                for kt in range(KT):
                    # first 512 columns only -- all chunk-0 needs; the rest
                    # streams during chunk-0 compute
                    nc.sync.dma_start(
                        out=ht0[kt][:, 0:512], in_=hT[kt * 128 : (kt + 1) * 128, 0:512]
                    )
                    arrival.append(("ht", kt))
                    if kt == 1:
                        nc.sync.dma_start(
                            out=w_sb[0][:, 8 * 128 :], in_=wqk[0][:, 8 * 128 :]
                        )
                        arrival.append(("whi", 0))
                    if kt % 3 == 2 and kt // 3 < 5:
                        j = kt // 3 + 1
                        nc.sync.dma_start(
                            out=w_sb[j][:, : 8 * 128], in_=wqk[j][:, : 8 * 128]
                        )
                        arrival.append(("wlo", j))
                    if kt % 3 == 0 and kt > 0 and kt // 3 < 6:
                        j = kt // 3
                        nc.sync.dma_start(
                            out=w_sb[j][:, 8 * 128 :], in_=wqk[j][:, 8 * 128 :]
                        )
                        arrival.append(("whi", j))
                for kt in range(KT):
                    nc.sync.dma_start(
                        out=ht0[kt][:, 512:1024],
                        in_=hT[kt * 128 : (kt + 1) * 128, 512:1024],
                    )
                nc.sync.dma_start(out=rotp_sb[:], in_=rotp[:])
                nc.sync.dma_start(out=ident_sb[:], in_=identb[:])
                nc.sync.dma_start(out=tri_sb[:], in_=trimask[:])
                nc.sync.dma_start(out=cos_sb[:], in_=cosT[:])
                nc.sync.dma_start(out=sin_sb[:], in_=sinT[:])
                for kt in range(KT):
                    nc.sync.dma_start(
                        out=ht1[kt][:], in_=hT[kt * 128 : (kt + 1) * 128, 1024:2048]
                    )
                for kb in range(4):
                    nc.sync.dma_start(out=wo_sb[kb][:], in_=wo[kb])

                # ---- Phase 1 half 0 ----
                # psrot spans phase 1 and the half-1 filler (manual scope:
                # closed right after the psqB block below)
                psrot_cm = tc.tile_pool(name="psrot", bufs=1, space="PSUM")
                psrot = psrot_cm.__enter__()
                psvt_cm = tc.tile_pool(name="psvt", bufs=1, space="PSUM")
                psvt = psvt_cm.__enter__()
                with tc.tile_pool(name="psqA", bufs=6, space="PSUM") as psqA:
                    for c2 in range(2):
                        _mark(nc, f'p1h0-c{c2}')
                        cs = c2 * 512
                        ps = [
                            psqA.tile([128, 512], F32, name=f"psq{i}", tag="psq")
                            for i in range(6)
                        ]

                        def mm(i, kt):
                            nc.tensor.matmul(
                                ps[i][:],
                                lhsT=w_sb[i][:, kt * 128 : (kt + 1) * 128],
                                rhs=ht0[kt][:, c2 * 512 : (c2 + 1) * 512],
                                start=(kt == 0),
                                stop=(kt == KT - 1),
                            )

                        if c2 == 0:
                            # wavefront emission matching DMA arrival order
                            # per-chain arrived-kt watermark: lo half = kt<8,
                            # hi half completes the chain
                            wlim = [0] * 6
                            akt = 0
                            emitted = [0] * 6

                            def advance():
                                for i in range(6):
                                    top = min(akt, wlim[i])
                                    for kt in range(emitted[i], top):
                                        mm(i, kt)
                                    emitted[i] = max(emitted[i], top)

                            for kind, idx in arrival:
                                if kind == "w2":
                                    wlim[idx] = 2
                                elif kind == "wlo":
                                    wlim[idx] = 8
                                elif kind == "whi":
                                    wlim[idx] = 16
                                else:
                                    akt = idx + 1
                                advance()
                            _mark(nc, f'p1h0-consume-c{c2}')
                            # k/v consumes first: attention's first scores
                            # need kT and vaug complete (tile-granular deps)
                            for i in (4, 5, 0, 1, 2, 3):
                                _consume_proj(
                                    nc, sb1, psrot, psvt, rotp_sb, ident_sb,
                                    ps[i], i, cs, qk, vaug, cos_sb, sin_sb,
                                    use_act=(i % 2 == 0),
                                )
                        else:
                            # chain-outer with k/v chains first so their
                            # consumes (kT add, vaug transposes) finish well
                            # before the PE drains the remaining q chains --
                            # the attention phase then starts with no stall
                            _mark(nc, f'p1h0-consume-c{c2}')
                            for i in (4, 5, 0, 1, 2, 3):
                                for kt in range(KT):
                                    mm(i, kt)
                                _consume_proj(
                                    nc, sb1, psrot, psvt, rotp_sb, ident_sb,
                                    ps[i], i, cs, qk, vaug, cos_sb, sin_sb,
                                    use_act=(i % 2 == 0),
                                )

                # ---- Phase 2+3 (attention; half-1 QKV and o-proj fill) ----
                if True:
                    pending = []  # o-proj chunks (sb, n) ready to emit
                    state = {"gen": None, "psop": None, "sc": None,
                             "scw": None, "pv": None}
                    clock = {"pe": 0.0, "act": 0.0}

                    def emit_oproj_chunk(final=False):
                        sb, n = pending.pop(0)
                        _mark(nc, f'oproj-{sb}-{n}')
                        pst = state["psop"].tile([128, 512], F32, tag="po")
                        for kb in range(4):
                            nc.tensor.matmul(
                                pst[:],
                                lhsT=attnT[kb][:, sb * 128 : (sb + 1) * 128],
                                rhs=wo_sb[kb][:, n * 512 : (n + 1) * 512],
                                start=(kb == 0),
                                stop=(kb == 3),
                            )
                        # accumulate the four 512-col chunks of one output
                        # row-block in SBUF, write the row with ONE DMA --
                        # chunks of an sb are popped consecutively (FIFO)
                        if n == 0:
                            state["orow"] = sp.tile(
                                [128, H], BF16, name="orow", tag="orow"
                            )
                        if final and n % 2 == 1:
                            nc.scalar.copy(
                                state["orow"][:, n * 512 : (n + 1) * 512], pst[:]
                            )
                        else:
                            nc.vector.tensor_copy(
                                state["orow"][:, n * 512 : (n + 1) * 512], pst[:]
                            )
                        if n == 3:
                            nc.sync.dma_start(
                                out=out[sb * 128 : (sb + 1) * 128, :],
                                in_=state["orow"][:],
                            )

                    def tick(pe_ns=0.0, act_ns=0.0):
                        clock["pe"] += pe_ns
                        clock["act"] += act_ns
                        while clock["act"] > clock["pe"] + 400.0:
                            if state["gen"] is not None:
                                try:
                                    clock["pe"] += next(state["gen"])
                                    continue
                                except StopIteration:
                                    state["gen"] = None
                            if pending and state["psop"] is not None:
                                emit_oproj_chunk()
                                clock["pe"] += 524.0
                                continue
                            break

                    kT = qk[4]

                    def emit_qb(qb):
                        q0 = qb * NQ
                        nj = 4 * qb + 4
                        for h in range(4):
                            _mark(nc, f'qb{qb}-h{h}')
                            qT = qk[h]
                            e_tiles = {}

                            def emit_j(j):
                                r4 = j - 4 * qb
                                off = max(0, r4) * 128
                                w = NQ - off
                                sps = state["sc"].tile([128, NQ], F32, tag="sc")
                                nc.tensor.matmul(
                                    sps[:, off:NQ],
                                    lhsT=kT[:, j * 128 : (j + 1) * 128],
                                    rhs=qT[:, q0 + off : q0 + NQ],
                                    start=True,
                                    stop=True,
                                )
                                e = ep.tile([128, NQ], BF16, tag="e")
                                nc.scalar.activation(
                                    e[:, off:NQ], sps[:, off:NQ], AF.Exp, scale=SCALE
                                )
                                if r4 >= 0:
                                    nc.gpsimd.tensor_mul(
                                        e[:, off : off + 128],
                                        e[:, off : off + 128],
                                        tri_sb[:],
                                    )
                                e_tiles[j] = e
                                tick(pe_ns=w / 4.8 + 35.0, act_ns=w * 0.8333 + 290.0)

                            def emit_j_pair(j0, j1):
                                # two full-width score tiles into one 2-bank
                                # psum (each matmul stays within one bank),
                                # ONE exp over [128, 1024] -- halves the
                                # per-instruction ACT overhead (~293ns each)
                                spw = state["scw"].tile(
                                    [128, 1024], F32, name="spw", tag="scw"
                                )
                                for k, j in enumerate((j0, j1)):
                                    nc.tensor.matmul(
                                        spw[:, k * 512 : (k + 1) * 512],
                                        lhsT=kT[:, j * 128 : (j + 1) * 128],
                                        rhs=qT[:, q0 : q0 + NQ],
                                        start=True,
                                        stop=True,
                                    )
                                e2 = ep.tile(
                                    [128, 1024], BF16, name="e2", tag="e2", bufs=7
                                )
                                nc.scalar.activation(e2[:], spw[:], AF.Exp, scale=SCALE)
                                e_tiles[j0] = e2[:, 0:512]
                                e_tiles[j1] = e2[:, 512:1024]
                                tick(
                                    pe_ns=2 * (NQ / 4.8 + 35.0),
                                    act_ns=1024 * 0.8333 + 290.0,
                                )

                            if qb >= 2 and state["scw"] is not None:
                                for a in range(0, 4 * qb, 2):
                                    emit_j_pair(a, a + 1)
                                for j in range(4 * qb, nj):
                                    emit_j(j)
                            else:
                                for j in range(nj):
                                    emit_j(j)

                            # PV: sequential per-t chains; vaug's ones column
                            # makes column 128 the softmax denominator
                            _mark(nc, f'qb{qb}-h{h}-pv')
                            rec = sp.tile([128, 4], F32, tag="rec")
                            asb = sp.tile([128, NQ], BF16, tag="asb")
                            for t in range(4):
                                # full-bank tile: psum slots pack at byte
                                # granularity, and a 516B slot would share
                                # its bank with the other buf -- start=True
                                # clears the whole bank's has_written bits,
                                # wiping the other chain's accumulation
                                pvt = state["pv"].tile([128, 512], F32, tag="pvt")
                                for j in range(4 * qb + t + 1):
                                    nc.tensor.matmul(
                                        pvt[:, 0:129],
                                        lhsT=e_tiles[j][:, t * 128 : (t + 1) * 128],
                                        rhs=vaug[:, j * 136 : j * 136 + 129],
                                        start=(j == 0),
                                        stop=(j == 4 * qb + t),
                                    )
                                tick(pe_ns=(4 * qb + t + 1) * 81.0)
                                nc.vector.reciprocal(rec[:, t : t + 1], pvt[:, 128:129])
                                nc.vector.tensor_scalar_mul(
                                    asb[:, t * 128 : (t + 1) * 128],
                                    pvt[:, 0:128],
                                    rec[:, t : t + 1],
                                )
                            # ONE merged DMA-xbar transpose for all four t
                            # blocks: the 3D-out form block-transposes
                            # (out[p,b,x] = in[x, b*128+p]), so
                            # attnT[d, q0+t*128+sq] = asb[sq, t*128+d]
                            # directly -- 4x fewer DMA instructions (each
                            # costs ~600ns serialized HWDGE + 900ns sem)
                            nc.sync.dma_start_transpose(
                                attnT[h][:, q0 : q0 + NQ].rearrange(
                                    "p (b x) -> p b x", b=4
                                ),
                                asb[:],
                            )
                            e_tiles.clear()

                        for sbl in range(4):
                            for n in range(4):
                                pending.append((qb * 4 + sbl, n))

                    # qb0/qb1 share psum with half-1 QKV (i-outer, 2 bufs)
                    with (
                        tc.tile_pool(name="sps", bufs=3, space="PSUM") as spsp,
                        tc.tile_pool(name="pv", bufs=1, space="PSUM") as pvp,
                        tc.tile_pool(name="psqB", bufs=2, space="PSUM") as psqB,
                    ):
                        state["sc"] = spsp
                        state["pv"] = pvp

                        def qkv_half1():
                            for c2 in range(2):
                                cs = 1024 + c2 * 512
                                for i in range(6):
                                    _mark(nc, f'p1h1-c{c2}-i{i}')
                                    p = psqB.tile([128, 512], F32, tag="psqB")
                                    for kt0 in range(0, KT, 4):
                                        for kt in range(kt0, kt0 + 4):
                                            nc.tensor.matmul(
                                                p[:],
                                                lhsT=w_sb[i][:, kt * 128 : (kt + 1) * 128],
                                                rhs=ht1[kt][:, c2 * 512 : (c2 + 1) * 512],
                                                start=(kt == 0),
                                                stop=(kt == KT - 1),
                                            )
                                        yield 4 * 112.0
                                    _consume_proj(
                                        nc, sb1, psrot, psvt, rotp_sb, ident_sb,
                                        p, i, cs, qk, vaug, cos_sb, sin_sb,
                                        use_act=False,
                                    )
                                    yield 250.0

                        state["gen"] = qkv_half1()
                        emit_qb(0)
                        emit_qb(1)
                        g = state["gen"]
                        if g is not None:
                            for _ in g:
                                pass
                            state["gen"] = None

                psvt_cm.__exit__(None, None, None)
                psrot_cm.__exit__(None, None, None)

                # qb2/qb3: all 8 banks free -- wide 2-bank score psums for
                # the paired exps, 1 narrow for the causal-diagonal tiles
                with (
                    tc.tile_pool(name="scw", bufs=2, space="PSUM") as scwp,
                    tc.tile_pool(name="sps2", bufs=1, space="PSUM") as sps2,
                    tc.tile_pool(name="pv2", bufs=1, space="PSUM") as pvp2,
                    tc.tile_pool(name="pso", bufs=2, space="PSUM") as psop,
                ):
                    state["sc"] = sps2
                    state["scw"] = scwp
                    state["pv"] = pvp2
                    state["psop"] = psop
                    emit_qb(2)
                    emit_qb(3)
                    _mark(nc, 'final-drain')
                    while pending:
                        emit_oproj_chunk(final=True)

    nc.compile()
    return nc


def _consume_proj(nc, sb1, psrot, psvt, rotp_sb, ident_sb, p, i, cs, qk, vaug,
                  cos_sb, sin_sb, use_act=True):
    """Consume projection psum p (head-block i) for s-chunk [cs, cs+512)."""
    if i < 5:
        qraw = sb1.tile([128, 512], BF16, tag="qraw")
        # split psum-drain copies between scalar and vector engines
        if use_act:
            nc.scalar.copy(qraw[:], p[:])
        else:
            nc.vector.tensor_copy(qraw[:], p[:])
        rot = psrot.tile([128, 512], F32, tag="rot")
        nc.tensor.matmul(rot[:], lhsT=rotp_sb[:], rhs=qraw[:], start=True, stop=True)
        tmp = sb1.tile([128, 512], F32, tag="tmp")
        nc.vector.tensor_mul(tmp[:], rot[:], sin_sb[:, cs : cs + 512])
        t2 = sb1.tile([128, 512], F32, tag="t2")
        nc.gpsimd.tensor_mul(t2[:], qraw[:], cos_sb[:, cs : cs + 512])
        nc.vector.tensor_add(qk[i][:, cs : cs + 512], t2[:], tmp[:])
    else:
        vTc = sb1.tile([128, 512], BF16, tag="vTc")
        nc.vector.tensor_copy(vTc[:], p[:])
        for sbl in range(4):
            sb = cs // 128 + sbl
            pv = psvt.tile([128, 128], BF16, tag="psv")
            nc.tensor.transpose(pv[:], vTc[:, sbl * 128 : (sbl + 1) * 128], ident_sb[:])
            if use_act:
                nc.scalar.copy(vaug[:, sb * 136 : sb * 136 + 128], pv[:])
            else:
                nc.vector.tensor_copy(vaug[:, sb * 136 : sb * 136 + 128], pv[:])
    return


def _prep_inputs(hidden_states, cos, sin, w_qkv, w_o):
    """Build the 8 per-core input maps (host-side shard + transpose)."""
    bf = ml_dtypes.bfloat16
    hidden_states = np.asarray(hidden_states, dtype=np.float32)
    cos = np.asarray(cos, dtype=np.float32)
    sin = np.asarray(sin, dtype=np.float32)
    w_qkv = np.asarray(w_qkv, dtype=np.float32)
    w_o = np.asarray(w_o, dtype=np.float32)

    cosT = np.ascontiguousarray(cos.T).astype(bf)
    sinT = np.ascontiguousarray(sin.T).copy()
    sinT[0:64] *= -1.0  # rotate_half sign folded into sin

    d = np.arange(128)
    rotp = ((d[None, :] == (d[:, None] + 64) % 128)).astype(bf)
    identb = np.eye(128).astype(bf)
    sk = np.arange(128)[:, None]
    c = np.arange(128)[None, :]
    trimask = (sk <= c).astype(bf)

    hT = [np.ascontiguousarray(hidden_states[b].T).astype(bf) for b in range(2)]

    in_maps = []
    for cidx in range(8):
        b, g = divmod(cidx, 4)
        W6 = np.stack(
            [w_qkv[(4 * g + i) * 128 : (4 * g + i + 1) * 128] for i in range(4)]
            + [w_qkv[(16 + g) * 128 : (17 + g) * 128]]
            + [w_qkv[(20 + g) * 128 : (21 + g) * 128]]
        )  # [6, 128 m, 2048 h]
        wqk_pack = np.ascontiguousarray(
            W6.transpose(0, 2, 1)  # [6, h, m]
            .reshape(6, KT, 128, 128)  # [6, kt, p, m]
            .transpose(0, 2, 1, 3)  # [6, p, kt, m]
            .reshape(6, 128, KT * 128)
        ).astype(bf)
        wo_pack = np.ascontiguousarray(
            np.stack(
                [
                    w_o[:, (4 * g + kb) * 128 : (4 * g + kb + 1) * 128].T
                    for kb in range(4)
                ]
            )
        ).astype(bf)  # [4, 128 hd, 2048 o]
        in_maps.append(
            dict(
                hT=hT[b],
                wqk=wqk_pack,
                cosT=cosT,
                sinT=sinT,
                rotp=rotp,
                identb=identb,
                trimask=trimask,
                wo=wo_pack,
            )
        )
    return in_maps


def run(hidden_states, cos, sin, w_qkv, w_o, trace=False, **trace_kwargs):
    if "nc" not in _CACHED:
        _CACHED["nc"] = build_nc()
    nc = _CACHED["nc"]
    in_maps = _prep_inputs(hidden_states, cos, sin, w_qkv, w_o)
    res = run_bass_kernel_spmd(
        nc, in_maps, core_ids=list(range(8)), trace=trace, **trace_kwargs
    )
    outs = [res.results[c]["out"].astype(np.float32) for c in range(8)]
    full = np.stack(
        [
            outs[0] + outs[1] + outs[2] + outs[3],
            outs[4] + outs[5] + outs[6] + outs[7],
        ]
    ).astype(np.float32)
    return full, res


def kernel(hidden_states, cos, sin, w_qkv, w_o):
    full, _ = run(hidden_states, cos, sin, w_qkv, w_o, trace=False)
    return full



# revision 46
# speedup vs baseline: 1.1907x; 1.1330x over previous
"""Trainium2 Bass kernel for GQA attention block (B=2, S=2048, H=2048,
16 q-heads / 4 kv-heads, head_dim=128, RoPE, causal) on 8 NeuronCores.

Sharding: core c -> batch b = c // 4, kv-group g = c % 4
  (q heads 4g..4g+3, kv head g).  Each core computes its batch's
  attention for its 4 query heads plus the partial output projection
  over its 512 hidden columns of w_o; host sums the 4 partials per batch.

Design (all matmuls bf16, PSUM accumulate f32):
  Phase 1  QKV projection; half 0: chunk c2=0 runs as a 6-chain wavefront
           matched to the DMA arrival order (first w tile split so the
           first matmul starts after ~100KB of DMA); chunk c2=1 runs
           chain-outer with the k/v chains FIRST so their consumes (kT
           RoPE add, v transposes into vaug) are complete before the PE
           drains the remaining q chains -- attention then starts with no
           stall.  Half 1 is emitted lazily as PE *filler* inside the
           qb0/qb1 attention loops (i-outer, 2 psum bufs, 4-matmul
           segments).  RoPE rotate-half via a PE permutation matmul, v
           transposed on PE into vaug blocks (136-col stride; col 128 =
           ones).
  Phase 2  per query block (512) x head: scores^T [sk part, sq free],
           exp on scalar engine, causal diagonal as a post-exp 0/1
           triangle multiply on gpsimd, then PV with e as stationary
           operand: out[sq, 129] = e^T @ [v | 1] -- the 129th column is
           the softmax denominator (free).  PV psum padded to a full
           2KB bank (sub-bank psum slots share a bank and matmul
           start=True clears the whole bank's has_written bits).
           Per-partition reciprocal + tensor_scalar normalize, DMA-xbar
           transpose back to attnT [d, s].
  Phase 3  o-proj chunks drained by a deficit pacer (emitted-PE-ns vs
           emitted-scalar-ns clocks) into later query blocks' attention
           so the PE never starves while the scalar engine churns exps.
           The four 512-col chunks of each output row-block accumulate
           in one SBUF row tile and leave in a single [128, 2048] DMA
           (4x fewer out-DMAs -- each DMA instruction costs ~600ns of
           serialized HWDGE plus ~900ns of semaphore propagation).
"""

import contextlib
import math
import numpy as np
import ml_dtypes

import concourse.bacc as bacc
import concourse.mybir as mybir
import concourse.tile as tile
from concourse.bass_utils import run_bass_kernel_spmd

F32 = mybir.dt.float32
BF16 = mybir.dt.bfloat16
AF = mybir.ActivationFunctionType

S = 2048
H = 2048
D = 128            # head dim
KT = 16            # contraction tiles over hidden (2048/128)
NQ = 512           # query block width in attention
SCALE = 1.0 / math.sqrt(D)

_CACHED = {}
MARKS = []  # (label, instruction-counter) build-time phase markers


def _mark(nc, label):
    MARKS.append((label, len(nc._state.inst_map)))


def build_nc(loop_n=None):
    MARKS.clear()
    nc = bacc.Bacc(None, target_bir_lowering=False)
    hT = nc.dram_tensor("hT", [H, S], BF16, kind="ExternalInput")
    wqk = nc.dram_tensor("wqk", [6, 128, KT * 128], BF16, kind="ExternalInput")
    cosT = nc.dram_tensor("cosT", [D, S], BF16, kind="ExternalInput")
    sinT = nc.dram_tensor("sinT", [D, S], F32, kind="ExternalInput")
    rotp = nc.dram_tensor("rotp", [128, 128], BF16, kind="ExternalInput")
    identb = nc.dram_tensor("identb", [128, 128], BF16, kind="ExternalInput")
    trimask = nc.dram_tensor("trimask", [128, 128], BF16, kind="ExternalInput")
    wo = nc.dram_tensor("wo", [4, 128, H], BF16, kind="ExternalInput")
    out = nc.dram_tensor("out", [S, H], BF16, kind="ExternalOutput")

    with tile.TileContext(nc) as tc:
        with tc.tile_pool(name="persist", bufs=1) as pp:
          with (tc.For_i(0, loop_n, 1) if loop_n else contextlib.nullcontext()):
            # ---- persistent tiles ----
            qk = [pp.tile([128, S], BF16, name=f"qk{i}", tag=f"qk{i}") for i in range(5)]
            # v blocks padded to 136 cols so each block start is 16B-aligned
            # (the DMA xbar transpose writes require it); col 128 = ones
            vaug = pp.tile([128, 16 * 136], BF16, tag="vaug")
            cos_sb = pp.tile([128, S], BF16, tag="cos")
            sin_sb = pp.tile([128, S], F32, tag="sin")
            rotp_sb = pp.tile([128, 128], BF16, tag="rotp")
            ident_sb = pp.tile([128, 128], BF16, tag="ident")
            tri_sb = pp.tile([128, 128], BF16, tag="tri")
            attnT = [
                pp.tile([128, S], BF16, name=f"at{h}", tag=f"at{h}") for h in range(4)
            ]
            wo_sb = [
                pp.tile([128, H], BF16, name=f"wo{kb}", tag=f"wo{kb}") for kb in range(4)
            ]

            nc.vector.memset(vaug[:, 128::136], 1.0)
            # warm the scalar engine's exp table while the PE waits on DMA
            scratch = pp.tile([1, 8], F32, tag="scratch")
            nc.vector.memset(scratch[:], 0.0)
            nc.scalar.activation(scratch[:], scratch[:], AF.Exp, scale=1.0)

            # ---- SBUF pools spanning both phases ----
            with (
                tc.tile_pool(name="ht", bufs=1) as htp,
                tc.tile_pool(name="wq", bufs=1) as wqp,
                tc.tile_pool(name="p1sb", bufs=2) as sb1,
                tc.tile_pool(name="epool", bufs=10) as ep,  # e2 bufs set per-call
                tc.tile_pool(name="small", bufs=2) as sp,
            ):
                w_sb = [
                    wqp.tile([128, KT * 128], BF16, name=f"w{i}", tag=f"w{i}")
                    for i in range(6)
                ]
                ht0 = [
                    htp.tile([128, 1024], BF16, name=f"ht{kt}", tag=f"ht{kt}")
                    for kt in range(KT)
                ]
                ht1 = [
                    htp.tile([128, 1024], BF16, name=f"htb{kt}", tag=f"htb{kt}")
                    for kt in range(KT)
                ]
                # DMA queue: half-0 feed (weights interleaved in consumption
                # order), then non-PE-blocking small tiles, then half-1 ht
                # (prefetch during half-0 compute), then wo.
                arrival = []
                # first two kt-tiles of w0 alone so the first matmul can
                # start after ~100KB of DMA instead of ~380KB
                nc.sync.dma_start(out=w_sb[0][:, : 2 * 128], in_=wqk[0][:, : 2 * 128])
                arrival.append(("w2", 0))
                nc.sync.dma_start(
                    out=w_sb[0][:, 2 * 128 : 8 * 128], in_=wqk[0][:, 2 * 128 : 8 * 128]
                )
                arrival.append(("wlo", 0))
                for kt in range(KT):
                    # first 512 columns only -- all chunk-0 needs; the rest
                    # streams during chunk-0 compute
                    nc.sync.dma_start(
                        out=ht0[kt][:, 0:512], in_=hT[kt * 128 : (kt + 1) * 128, 0:512]
                    )
                    arrival.append(("ht", kt))
                    if kt == 1:
                        nc.sync.dma_start(
                            out=w_sb[0][:, 8 * 128 :], in_=wqk[0][:, 8 * 128 :]
                        )
                        arrival.append(("whi", 0))
                    if kt % 3 == 2 and kt // 3 < 5:
                        j = kt // 3 + 1
                        nc.sync.dma_start(
                            out=w_sb[j][:, : 8 * 128], in_=wqk[j][:, : 8 * 128]
                        )
                        arrival.append(("wlo", j))
                    if kt % 3 == 0 and kt > 0 and kt // 3 < 6:
                        j = kt // 3
                        nc.sync.dma_start(
                            out=w_sb[j][:, 8 * 128 :], in_=wqk[j][:, 8 * 128 :]
                        )
                        arrival.append(("whi", j))
                for kt in range(KT):
                    nc.sync.dma_start(
                        out=ht0[kt][:, 512:1024],
                        in_=hT[kt * 128 : (kt + 1) * 128, 512:1024],
                    )
                nc.sync.dma_start(out=rotp_sb[:], in_=rotp[:])
                nc.sync.dma_start(out=ident_sb[:], in_=identb[:])
                nc.sync.dma_start(out=tri_sb[:], in_=trimask[:])
                nc.sync.dma_start(out=cos_sb[:], in_=cosT[:])
                nc.sync.dma_start(out=sin_sb[:], in_=sinT[:])
                for kt in range(KT):
                    nc.sync.dma_start(
                        out=ht1[kt][:], in_=hT[kt * 128 : (kt + 1) * 128, 1024:2048]
                    )
                for kb in range(4):
                    nc.sync.dma_start(out=wo_sb[kb][:], in_=wo[kb])

                # ---- Phase 1 half 0 ----
                # psrot spans phase 1 and the half-1 filler (manual scope:
                # closed right after the psqB block below)
                psrot_cm = tc.tile_pool(name="psrot", bufs=1, space="PSUM")
                psrot = psrot_cm.__enter__()
                psvt_cm = tc.tile_pool(name="psvt", bufs=1, space="PSUM")
                psvt = psvt_cm.__enter__()
                with tc.tile_pool(name="psqA", bufs=6, space="PSUM") as psqA:
                    for c2 in range(2):
                        _mark(nc, f'p1h0-c{c2}')
                        cs = c2 * 512
                        ps = [
                            psqA.tile([128, 512], F32, name=f"psq{i}", tag="psq")
                            for i in range(6)
                        ]

                        def mm(i, kt):
                            nc.tensor.matmul(
                                ps[i][:],
                                lhsT=w_sb[i][:, kt * 128 : (kt + 1) * 128],
                                rhs=ht0[kt][:, c2 * 512 : (c2 + 1) * 512],
                                start=(kt == 0),
                                stop=(kt == KT - 1),
                            )

                        if c2 == 0:
                            # wavefront emission matching DMA arrival order
                            # per-chain arrived-kt watermark: lo half = kt<8,
                            # hi half completes the chain
                            wlim = [0] * 6
                            akt = 0
                            emitted = [0] * 6

                            def advance():
                                for i in range(6):
                                    top = min(akt, wlim[i])
                                    for kt in range(emitted[i], top):
                                        mm(i, kt)
                                    emitted[i] = max(emitted[i], top)

                            for kind, idx in arrival:
                                if kind == "w2":
                                    wlim[idx] = 2
                                elif kind == "wlo":
                                    wlim[idx] = 8
                                elif kind == "whi":
                                    wlim[idx] = 16
                                else:
                                    akt = idx + 1
                                advance()
                            _mark(nc, f'p1h0-consume-c{c2}')
                            # k/v consumes first: attention's first scores
                            # need kT and vaug complete (tile-granular deps)
                            for i in (4, 5, 0, 1, 2, 3):
                                _consume_proj(
                                    nc, sb1, psrot, psvt, rotp_sb, ident_sb,
                                    ps[i], i, cs, qk, vaug, cos_sb, sin_sb,
                                    use_act=(i % 2 == 0),
                                )
                        else:
                            # chain-outer with k/v chains first so their
                            # consumes (kT add, vaug transposes) finish well
                            # before the PE drains the remaining q chains --
                            # the attention phase then starts with no stall
                            _mark(nc, f'p1h0-consume-c{c2}')
                            for i in (4, 5, 0, 1, 2, 3):
                                for kt in range(KT):
                                    mm(i, kt)
                                _consume_proj(
                                    nc, sb1, psrot, psvt, rotp_sb, ident_sb,
                                    ps[i], i, cs, qk, vaug, cos_sb, sin_sb,
                                    use_act=(i % 2 == 0),
                                )

                # ---- Phase 2+3 (attention; half-1 QKV and o-proj fill) ----
                if True:
                    pending = []  # o-proj chunks (sb, n) ready to emit
                    state = {"gen": None, "psop": None, "sc": None,
                             "scw": None, "pv": None}
                    clock = {"pe": 0.0, "act": 0.0}

                    def emit_oproj_chunk(final=False):
                        sb, n = pending.pop(0)
                        _mark(nc, f'oproj-{sb}-{n}')
                        pst = state["psop"].tile([128, 512], F32, tag="po")
                        for kb in range(4):
                            nc.tensor.matmul(
                                pst[:],
                                lhsT=attnT[kb][:, sb * 128 : (sb + 1) * 128],
                                rhs=wo_sb[kb][:, n * 512 : (n + 1) * 512],
                                start=(kb == 0),
                                stop=(kb == 3),
                            )
                        # accumulate the four 512-col chunks of one output
                        # row-block in SBUF, write the row with ONE DMA --
                        # chunks of an sb are popped consecutively (FIFO)
                        if n == 0:
                            state["orow"] = sp.tile(
                                [128, H], BF16, name="orow", tag="orow"
                            )
                        if final and n % 2 == 1:
                            nc.scalar.copy(
                                state["orow"][:, n * 512 : (n + 1) * 512], pst[:]
                            )
                        else:
                            nc.vector.tensor_copy(
                                state["orow"][:, n * 512 : (n + 1) * 512], pst[:]
                            )
                        if n == 3:
                            nc.sync.dma_start(
                                out=out[sb * 128 : (sb + 1) * 128, :],
                                in_=state["orow"][:],
                            )

                    def tick(pe_ns=0.0, act_ns=0.0):
                        clock["pe"] += pe_ns
                        clock["act"] += act_ns
                        while clock["act"] > clock["pe"] + 400.0:
                            if state["gen"] is not None:
                                try:
                                    clock["pe"] += next(state["gen"])
                                    continue
                                except StopIteration:
                                    state["gen"] = None
                            if pending and state["psop"] is not None:
                                emit_oproj_chunk()
                                clock["pe"] += 524.0
                                continue
                            break

                    kT = qk[4]

                    def emit_qb(qb):
                        q0 = qb * NQ
                        nj = 4 * qb + 4
                        for h in range(4):
                            _mark(nc, f'qb{qb}-h{h}')
                            qT = qk[h]
                            e_tiles = {}

                            def emit_j(j):
                                r4 = j - 4 * qb
                                off = max(0, r4) * 128
                                w = NQ - off
                                sps = state["sc"].tile([128, NQ], F32, tag="sc")
                                nc.tensor.matmul(
                                    sps[:, off:NQ],
                                    lhsT=kT[:, j * 128 : (j + 1) * 128],
                                    rhs=qT[:, q0 + off : q0 + NQ],
                                    start=True,
                                    stop=True,
                                )
                                e = ep.tile([128, NQ], BF16, tag="e")
                                nc.scalar.activation(
                                    e[:, off:NQ], sps[:, off:NQ], AF.Exp, scale=SCALE
                                )
                                if r4 >= 0:
                                    nc.gpsimd.tensor_mul(
                                        e[:, off : off + 128],
                                        e[:, off : off + 128],
                                        tri_sb[:],
                                    )
                                e_tiles[j] = e
                                tick(pe_ns=w / 4.8 + 35.0, act_ns=w * 0.8333 + 290.0)

                            def emit_j_pair(j0, j1):
                                # two full-width score tiles into one 2-bank
                                # psum (each matmul stays within one bank),
                                # ONE exp over [128, 1024] -- halves the
                                # per-instruction ACT overhead (~293ns each)
                                spw = state["scw"].tile(
                                    [128, 1024], F32, name="spw", tag="scw"
                                )
                                for k, j in enumerate((j0, j1)):
                                    nc.tensor.matmul(
                                        spw[:, k * 512 : (k + 1) * 512],
                                        lhsT=kT[:, j * 128 : (j + 1) * 128],
                                        rhs=qT[:, q0 : q0 + NQ],
                                        start=True,
                                        stop=True,
                                    )
                                e2 = ep.tile(
                                    [128, 1024], BF16, name="e2", tag="e2", bufs=7
                                )
                                nc.scalar.activation(e2[:], spw[:], AF.Exp, scale=SCALE)
                                e_tiles[j0] = e2[:, 0:512]
                                e_tiles[j1] = e2[:, 512:1024]
                                tick(
                                    pe_ns=2 * (NQ / 4.8 + 35.0),
                                    act_ns=1024 * 0.8333 + 290.0,
                                )

                            if qb >= 1 and state["scw"] is not None:
                                for a in range(0, 4 * qb, 2):
                                    emit_j_pair(a, a + 1)
                                for j in range(4 * qb, nj):
                                    emit_j(j)
                            else:
                                for j in range(nj):
                                    emit_j(j)

                            # PV: sequential per-t chains; vaug's ones column
                            # makes column 128 the softmax denominator
                            _mark(nc, f'qb{qb}-h{h}-pv')
                            rec = sp.tile([128, 4], F32, tag="rec")
                            asb = sp.tile([128, NQ], BF16, tag="asb")
                            for t in range(4):
                                # full-bank tile: psum slots pack at byte
                                # granularity, and a 516B slot would share
                                # its bank with the other buf -- start=True
                                # clears the whole bank's has_written bits,
                                # wiping the other chain's accumulation
                                pvt = state["pv"].tile([128, 512], F32, tag="pvt")
                                for j in range(4 * qb + t + 1):
                                    nc.tensor.matmul(
                                        pvt[:, 0:129],
                                        lhsT=e_tiles[j][:, t * 128 : (t + 1) * 128],
                                        rhs=vaug[:, j * 136 : j * 136 + 129],
                                        start=(j == 0),
                                        stop=(j == 4 * qb + t),
                                    )
                                tick(pe_ns=(4 * qb + t + 1) * 81.0)
                                nc.vector.reciprocal(rec[:, t : t + 1], pvt[:, 128:129])
                                nc.vector.tensor_scalar_mul(
                                    asb[:, t * 128 : (t + 1) * 128],
                                    pvt[:, 0:128],
                                    rec[:, t : t + 1],
                                )
                            # ONE merged DMA-xbar transpose for all four t
                            # blocks: the 3D-out form block-transposes
                            # (out[p,b,x] = in[x, b*128+p]), so
                            # attnT[d, q0+t*128+sq] = asb[sq, t*128+d]
                            # directly -- 4x fewer DMA instructions (each
                            # costs ~600ns serialized HWDGE + 900ns sem)
                            nc.sync.dma_start_transpose(
                                attnT[h][:, q0 : q0 + NQ].rearrange(
                                    "p (b x) -> p b x", b=4
                                ),
                                asb[:],
                            )
                            e_tiles.clear()

                        for sbl in range(4):
                            for n in range(4):
                                pending.append((qb * 4 + sbl, n))

                    # qb0/qb1 share psum with half-1 QKV (i-outer, 2 bufs)
                    with (
                        tc.tile_pool(name="scwA", bufs=1, space="PSUM") as scwA,
                        tc.tile_pool(name="sps", bufs=1, space="PSUM") as spsp,
                        tc.tile_pool(name="pv", bufs=1, space="PSUM") as pvp,
                        tc.tile_pool(name="psqB", bufs=2, space="PSUM") as psqB,
                    ):
                        state["sc"] = spsp
                        state["scw"] = scwA
                        state["pv"] = pvp

                        def qkv_half1():
                            for c2 in range(2):
                                cs = 1024 + c2 * 512
                                for i in range(6):
                                    _mark(nc, f'p1h1-c{c2}-i{i}')
                                    p = psqB.tile([128, 512], F32, tag="psqB")
                                    for kt0 in range(0, KT, 4):
                                        for kt in range(kt0, kt0 + 4):
                                            nc.tensor.matmul(
                                                p[:],
                                                lhsT=w_sb[i][:, kt * 128 : (kt + 1) * 128],
                                                rhs=ht1[kt][:, c2 * 512 : (c2 + 1) * 512],
                                                start=(kt == 0),
                                                stop=(kt == KT - 1),
                                            )
                                        yield 4 * 112.0
                                    _consume_proj(
                                        nc, sb1, psrot, psvt, rotp_sb, ident_sb,
                                        p, i, cs, qk, vaug, cos_sb, sin_sb,
                                        use_act=False,
                                    )
                                    yield 250.0

                        state["gen"] = qkv_half1()
                        emit_qb(0)
                        emit_qb(1)
                        g = state["gen"]
                        if g is not None:
                            for _ in g:
                                pass
                            state["gen"] = None

                psvt_cm.__exit__(None, None, None)
                psrot_cm.__exit__(None, None, None)

                # qb2/qb3: all 8 banks free -- wide 2-bank score psums for
                # the paired exps, 1 narrow for the causal-diagonal tiles
                with (
                    tc.tile_pool(name="scw", bufs=2, space="PSUM") as scwp,
                    tc.tile_pool(name="sps2", bufs=1, space="PSUM") as sps2,
                    tc.tile_pool(name="pv2", bufs=1, space="PSUM") as pvp2,
                    tc.tile_pool(name="pso", bufs=2, space="PSUM") as psop,
                ):
                    state["sc"] = sps2
                    state["scw"] = scwp
                    state["pv"] = pvp2
                    state["psop"] = psop
                    emit_qb(2)
                    emit_qb(3)
                    _mark(nc, 'final-drain')
                    while pending:
                        emit_oproj_chunk(final=True)

    nc.compile()
    return nc


def _consume_proj(nc, sb1, psrot, psvt, rotp_sb, ident_sb, p, i, cs, qk, vaug,
                  cos_sb, sin_sb, use_act=True):
    """Consume projection psum p (head-block i) for s-chunk [cs, cs+512)."""
    if i < 5:
        qraw = sb1.tile([128, 512], BF16, tag="qraw")
        # split psum-drain copies between scalar and vector engines
        if use_act:
            nc.scalar.copy(qraw[:], p[:])
        else:
            nc.vector.tensor_copy(qraw[:], p[:])
        rot = psrot.tile([128, 512], F32, tag="rot")
        nc.tensor.matmul(rot[:], lhsT=rotp_sb[:], rhs=qraw[:], start=True, stop=True)
        tmp = sb1.tile([128, 512], F32, tag="tmp")
        nc.vector.tensor_mul(tmp[:], rot[:], sin_sb[:, cs : cs + 512])
        t2 = sb1.tile([128, 512], F32, tag="t2")
        nc.gpsimd.tensor_mul(t2[:], qraw[:], cos_sb[:, cs : cs + 512])
        nc.vector.tensor_add(qk[i][:, cs : cs + 512], t2[:], tmp[:])
    else:
        vTc = sb1.tile([128, 512], BF16, tag="vTc")
        nc.vector.tensor_copy(vTc[:], p[:])
        for sbl in range(4):
            sb = cs // 128 + sbl
            pv = psvt.tile([128, 128], BF16, tag="psv")
            nc.tensor.transpose(pv[:], vTc[:, sbl * 128 : (sbl + 1) * 128], ident_sb[:])
            if use_act:
                nc.scalar.copy(vaug[:, sb * 136 : sb * 136 + 128], pv[:])
            else:
                nc.vector.tensor_copy(vaug[:, sb * 136 : sb * 136 + 128], pv[:])
    return


def _prep_inputs(hidden_states, cos, sin, w_qkv, w_o):
    """Build the 8 per-core input maps (host-side shard + transpose)."""
    bf = ml_dtypes.bfloat16
    hidden_states = np.asarray(hidden_states, dtype=np.float32)
    cos = np.asarray(cos, dtype=np.float32)
    sin = np.asarray(sin, dtype=np.float32)
    w_qkv = np.asarray(w_qkv, dtype=np.float32)
    w_o = np.asarray(w_o, dtype=np.float32)

    cosT = np.ascontiguousarray(cos.T).astype(bf)
    sinT = np.ascontiguousarray(sin.T).copy()
    sinT[0:64] *= -1.0  # rotate_half sign folded into sin

    d = np.arange(128)
    rotp = ((d[None, :] == (d[:, None] + 64) % 128)).astype(bf)
    identb = np.eye(128).astype(bf)
    sk = np.arange(128)[:, None]
    c = np.arange(128)[None, :]
    trimask = (sk <= c).astype(bf)

    hT = [np.ascontiguousarray(hidden_states[b].T).astype(bf) for b in range(2)]

    in_maps = []
    for cidx in range(8):
        b, g = divmod(cidx, 4)
        W6 = np.stack(
            [w_qkv[(4 * g + i) * 128 : (4 * g + i + 1) * 128] for i in range(4)]
            + [w_qkv[(16 + g) * 128 : (17 + g) * 128]]
            + [w_qkv[(20 + g) * 128 : (21 + g) * 128]]
        )  # [6, 128 m, 2048 h]
        wqk_pack = np.ascontiguousarray(
            W6.transpose(0, 2, 1)  # [6, h, m]
            .reshape(6, KT, 128, 128)  # [6, kt, p, m]
            .transpose(0, 2, 1, 3)  # [6, p, kt, m]
            .reshape(6, 128, KT * 128)
        ).astype(bf)
        wo_pack = np.ascontiguousarray(
            np.stack(
                [
                    w_o[:, (4 * g + kb) * 128 : (4 * g + kb + 1) * 128].T
                    for kb in range(4)
                ]
            )
        ).astype(bf)  # [4, 128 hd, 2048 o]
        in_maps.append(
            dict(
                hT=hT[b],
                wqk=wqk_pack,
                cosT=cosT,
                sinT=sinT,
                rotp=rotp,
                identb=identb,
                trimask=trimask,
                wo=wo_pack,
            )
        )
    return in_maps


def run(hidden_states, cos, sin, w_qkv, w_o, trace=False, **trace_kwargs):
    if "nc" not in _CACHED:
        _CACHED["nc"] = build_nc()
    nc = _CACHED["nc"]
    in_maps = _prep_inputs(hidden_states, cos, sin, w_qkv, w_o)
    res = run_bass_kernel_spmd(
        nc, in_maps, core_ids=list(range(8)), trace=trace, **trace_kwargs
    )
    outs = [res.results[c]["out"].astype(np.float32) for c in range(8)]
    full = np.stack(
        [
            outs[0] + outs[1] + outs[2] + outs[3],
            outs[4] + outs[5] + outs[6] + outs[7],
        ]
    ).astype(np.float32)
    return full, res


def kernel(hidden_states, cos, sin, w_qkv, w_o):
    full, _ = run(hidden_states, cos, sin, w_qkv, w_o, trace=False)
    return full

